# revision 1
# baseline (speedup 1.0000x reference)
"""GIN-style 7-layer GNN encoder on 8 Trainium2 NeuronCores.

Self-contained: kernel(**inputs) takes full numpy inputs, shards across 8
cores internally, runs a Bass/Tile kernel via run_bass_kernel_spmd, and
returns the full (4, 50000, 128) float32 output.

Architecture (per core, identical SPMD program; per-core data differs):
  - Nodes are degree-sorted and dealt round-robin to 8 cores (balances edge
    counts); each core owns TSLOT=6272 token slots (6250 real + 22 zero pads).
  - Token buffer in HBM: [50176, 128] f32, slot-major. Split at slot 31360
    into two halves so gather indices fit in int16 (dma_gather limit).
  - Per round: dma_gather pulls x[src] rows (512B each) into SBUF in 2048-idx
    calls; PE matmuls with small cached one-hot "staircase" patterns
    accumulate per-dst-node sums in PSUM (node-major); PE transposes to
    feature-major; 2-layer MLP on PE + ACT relu; BatchNorm stats on DVE with
    a tiny 8-core AllReduce; normalize; tokens written back + AllGather.
  - Rounds: layer 0, 1, 2, then one shared round for layers 3-6 (same input).
"""

import numpy as np

NCORES = 8
P = 128
D = 128
BN_EPS = 1e-5
CALL_IDX = 1024               # indices per dma_gather call (ring holds 128 descs/engine)
CPC = CALL_IDX // P           # chunks per call


# ----------------------------------------------------------------------------
# host-side schedule construction
# ----------------------------------------------------------------------------

class Plan:
    pass


def build_plan(edge_index: np.ndarray, n_nodes: int) -> Plan:
    """Build slot assignment, shared chunk schedule, per-core gather indices,
    and one-hot pattern pack from the (fixed) graph."""
    pl = Plan()
    src = edge_index[0].astype(np.int64)
    dst = edge_index[1].astype(np.int64)

    rslot = int(np.ceil(n_nodes / NCORES))          # real nodes per core
    tslot = int(np.ceil(rslot / P)) * P             # padded slots per core
    ntile = tslot // P
    tot = NCORES * tslot
    split = (NCORES // 2 + 1) * tslot               # half boundary (A = cores 0..4)
    assert split < 32768 and tot - split < 32768, (split, tot)

    deg = np.bincount(dst, minlength=n_nodes) + 1   # incl self edge

    order = np.argsort(-deg, kind="stable")         # global rank -> orig node
    # rank i -> core i%NCORES, local i//NCORES
    core_of = np.empty(n_nodes, np.int64)
    local_of = np.empty(n_nodes, np.int64)
    ranks = np.empty(n_nodes, np.int64)
    ranks[order] = np.arange(n_nodes)
    core_of = ranks % NCORES
    local_of = ranks // NCORES
    slot_of = core_of * tslot + local_of            # orig node -> global slot

    # orig node ids per core in local order (for output unshard)
    pl.orig_by_core = [order[c::NCORES] for c in range(NCORES)]

    # edge lists in CSR by (dst core, dst local), srcs as slots, self first
    dst_slot = slot_of[dst]
    src_slot = slot_of[src]
    sort_idx = np.argsort(dst_slot, kind="stable")
    ds_sorted = dst_slot[sort_idx]
    ss_sorted = src_slot[sort_idx]
    # counts per global slot
    cnt = np.bincount(ds_sorted, minlength=tot)
    offs = np.concatenate([[0], np.cumsum(cnt)])

    # per-(core, local) A/B degree incl self
    is_a_sorted = ss_sorted < split

    nA = np.zeros(tot, np.int64)
    nB = np.zeros(tot, np.int64)
    # vectorized per-slot half counts
    a_cnt = np.bincount(ds_sorted[is_a_sorted], minlength=tot)
    b_cnt = cnt - a_cnt
    self_is_a = (np.arange(tot) < split).astype(np.int64)
    nA = a_cnt + self_is_a
    nB = b_cnt + (1 - self_is_a)

    # shared (cross-core max) degree profile per tile position, min 1
    nA_c = nA.reshape(NCORES, tslot)
    nB_c = nB.reshape(NCORES, tslot)
    dmaxA = np.maximum(nA_c.max(axis=0), 1)         # [tslot]
    dmaxB = np.maximum(nB_c.max(axis=0), 1)
    assert dmaxA.max() <= P and dmaxB.max() <= P, (dmaxA.max(), dmaxB.max())

    # chunk packing per tile per half (shared schedule)
    # chunk: (half, tile, base_rr, m, dh, qidx) ; qidx = per-half chunk counter
    patterns = {}         # (dh, m) -> (pat_id, col_off)
    pat_cols = []         # list of (dh, m)
    pat_off = [0]

    def pack(dmax_t):
        chunks = []
        rr = 0
        while rr < P:
            m = 1
            dh = int(dmax_t[rr])
            while rr + m < P:
                nd = max(dh, int(dmax_t[rr + m]))
                if (m + 1) * nd <= P:
                    dh = nd
                    m += 1
                else:
                    break
            chunks.append((rr, m, dh))
            rr += m
        return chunks

    sched = []            # per tile: {"A": [(rr, m, dh, qidx)], "B": [...]}
    qcnt = {"A": 0, "B": 0}
    for t in range(ntile):
        ent = {}
        for half, dmax in (("A", dmaxA), ("B", dmaxB)):
            lst = []
            for (rr, m, dh) in pack(dmax[t * P:(t + 1) * P]):
                key = (dh, m)
                if key not in patterns:
                    patterns[key] = (len(pat_cols), pat_off[-1])
                    pat_cols.append(key)
                    pat_off.append(pat_off[-1] + m)
                lst.append((rr, m, dh, qcnt[half]))
                qcnt[half] += 1
            ent[half] = lst
        sched.append(ent)

    pl.sched = sched
    pl.nchunks = dict(qcnt)
    pl.ncalls = {h: int(np.ceil(qcnt[h] * P / CALL_IDX)) for h in "AB"}
    pl.pat_offsets = {k: patterns[k][1] for k in patterns}
    pl.pat_total_cols = pat_off[-1]

    # pattern pack matrix [P, pat_total_cols] f32: for (dh, m) at off:
    # col off+c has ones at rows c*dh .. (c+1)*dh-1
    packmat = np.zeros((P, pat_off[-1]), np.float32)
    for (dh, m), (pid, off) in patterns.items():
        for c in range(m):
            packmat[c * dh:(c + 1) * dh, off + c] = 1.0
    pl.packmat = packmat

    # per-core gather index arrays
    zslot_a = 0 * tslot + rslot                     # a pad slot of core 0
    zslot_b = (NCORES // 2 + 1) * tslot + rslot - split  # pad slot of core 5, rebased
    idx_arrays = {"A": [], "B": []}
    for c in range(NCORES):
        flat = {h: np.full(pl.ncalls[h] * CALL_IDX,
                           zslot_a if h == "A" else zslot_b, np.int16)
                for h in "AB"}
        base_slot = c * tslot
        for t in range(ntile):
            for half in "AB":
                zs = zslot_a if half == "A" else zslot_b
                rebase = 0 if half == "A" else split
                want_a = half == "A"
                for (rr, m, dh, q) in sched[t][half]:
                    out = flat[half]
                    o0 = q * P
                    for j in range(m):
                        loc = t * P + rr + j
                        gslot = base_slot + loc
                        row = o0 + j * dh
                        if loc < rslot:
                            # real node: its half edges
                            e0, e1 = offs[gslot], offs[gslot + 1]
                            ss = ss_sorted[e0:e1]
                            ss = ss[(ss < split) == want_a]
                            k = 0
                            if ((gslot < split) == want_a):
                                out[row] = gslot - rebase  # self edge
                                k = 1
                            n = len(ss)
                            out[row + k: row + k + n] = ss - rebase
                            # rest stays zslot
                        # pads: all zslot already
                    # rows beyond m*dh stay zslot
        for h in "AB":
            a = flat[h].reshape(-1, 16).T            # [16, ncalls*128]
            idx_arrays[h].append(np.tile(a, (8, 1)).astype(np.int16))
    pl.idxA = idx_arrays["A"]
    pl.idxB = idx_arrays["B"]

    pl.rslot, pl.tslot, pl.ntile, pl.tot, pl.split = rslot, tslot, ntile, tot, split
    pl.n_nodes = n_nodes
    slots_used = {h: qcnt[h] * P for h in "AB"}
    pl.inflation = (slots_used["A"] + slots_used["B"]) * NCORES / max(len(src) + n_nodes, 1)
    return pl


# ----------------------------------------------------------------------------
# bass kernel builder
# ----------------------------------------------------------------------------

def build_bass(pl: Plan):
    import os
    _NOCOLL = bool(os.environ.get("KDBG_NOCOLL"))
    _ROUNDS = int(os.environ.get("KDBG_ROUNDS", "4"))
    _NOGATHER = bool(os.environ.get("KDBG_NOGATHER"))
    _PHASE = int(os.environ.get("KDBG_PHASE", "99"))
    _REPS = int(os.environ.get("KDBG_REPS", "1"))
    import concourse.bacc as bacc
    import concourse.bass as bass
    import concourse.mybir as mybir
    import concourse.tile as tile
    from concourse.library_config import mlp as mlp_lib

    f32 = mybir.dt.float32
    i16 = mybir.dt.int16
    tslot, ntile, tot, split = pl.tslot, pl.ntile, pl.tot, pl.split
    rslot = pl.rslot
    ncallA, ncallB = pl.ncalls["A"], pl.ncalls["B"]
    NL = 7

    nc = bacc.Bacc("TRN2", target_bir_lowering=False, debug=False,
                   num_devices=NCORES)

    tok0 = nc.dram_tensor("tok0", [tot, D], f32, kind="ExternalInput")
    w1 = nc.dram_tensor("w1", [NL, D, D], f32, kind="ExternalInput")
    w2 = nc.dram_tensor("w2", [NL, D, D], f32, kind="ExternalInput")
    b1t = nc.dram_tensor("b1t", [D, NL], f32, kind="ExternalInput")
    b2t = nc.dram_tensor("b2t", [D, NL], f32, kind="ExternalInput")
    gt = nc.dram_tensor("gt", [D, NL], f32, kind="ExternalInput")
    bt = nc.dram_tensor("bt", [D, NL], f32, kind="ExternalInput")
    idxa_d = nc.dram_tensor("idxa", [128, ncallA * (CALL_IDX // 16)], i16,
                            kind="ExternalInput")
    idxb_d = nc.dram_tensor("idxb", [128, ncallB * (CALL_IDX // 16)], i16,
                            kind="ExternalInput")
    pat_d = nc.dram_tensor("pat", [P, pl.pat_total_cols], f32,
                           kind="ExternalInput")
    ident_d = nc.dram_tensor("ident", [P, P], f32, kind="ExternalInput")
    out_d = nc.dram_tensor("out", [4, D, tslot], f32, kind="ExternalOutput")

    with tile.TileContext(nc) as tc:
        nc.gpsimd.load_library(mlp_lib)
        with (
            tc.tile_pool(name="const", bufs=1) as constp,
            tc.tile_pool(name="ga", bufs=2) as gap,
            tc.tile_pool(name="gb", bufs=2) as gbp,
            tc.tile_pool(name="snm", bufs=2) as snmp,
            tc.tile_pool(name="st", bufs=1) as stp,
            tc.tile_pool(name="act", bufs=1) as actp,
            tc.tile_pool(name="relu", bufs=2) as relup,
            tc.tile_pool(name="toks", bufs=2) as tokp,
            tc.tile_pool(name="small", bufs=8) as smallp,
            tc.tile_pool(name="ps_s", bufs=3, space="PSUM") as ps_s,
            tc.tile_pool(name="ps_t", bufs=2, space="PSUM") as ps_t,
            tc.tile_pool(name="ps_mm", bufs=1, space="PSUM") as ps_mm,
            tc.tile_pool(name="dram", bufs=2, space="DRAM") as dramp,
        ):
            # ---- constants ----
            ident = constp.tile([P, P], f32)
            nc.sync.dma_start(ident[:], ident_d[:, :])
            pat = constp.tile([P, pl.pat_total_cols], f32)
            nc.sync.dma_start(pat[:], pat_d[:, :])
            idxa = constp.tile([128, ncallA * (CALL_IDX // 16)], i16)
            nc.sync.dma_start(idxa[:], idxa_d[:, :])
            idxb = constp.tile([128, ncallB * (CALL_IDX // 16)], i16)
            nc.sync.dma_start(idxb[:], idxb_d[:, :])
            w1sb = [constp.tile([D, D], f32, name=f"w1_{l}", tag=f"w1_{l}") for l in range(NL)]
            w2sb = [constp.tile([D, D], f32, name=f"w2_{l}", tag=f"w2_{l}") for l in range(NL)]
            for l in range(NL):
                nc.sync.dma_start(w1sb[l][:], w1[l, :, :])
                nc.sync.dma_start(w2sb[l][:], w2[l, :, :])
            b1sb = constp.tile([D, NL], f32)
            nc.sync.dma_start(b1sb[:], b1t[:, :])
            b2sb = constp.tile([D, NL], f32)
            nc.sync.dma_start(b2sb[:], b2t[:, :])
            gsb = constp.tile([D, NL], f32)
            nc.sync.dma_start(gsb[:], gt[:, :])
            btsb = constp.tile([D, NL], f32)
            nc.sync.dma_start(btsb[:], bt[:, :])
            epsc = constp.tile([D, 1], f32)
            nc.vector.memset(epsc[:], BN_EPS)
            zrhs = constp.tile([P, P], f32)
            nc.vector.memset(zrhs[:], 0.0)

            tok_src = tok0  # DRAM tensor handle for current round's tokens

            def gather_round(tok_ap_a, tok_ap_b):
                """Emit all gather calls; returns (gA_tiles, gB_tiles)."""
                gts = {"A": [], "B": []}
                for j in range(max(ncallA, ncallB)):
                    for half, ncall, idxt, src_ap, pool in (
                        ("A", ncallA, idxa, tok_ap_a, gap),
                        ("B", ncallB, idxb, tok_ap_b, gbp),
                    ):
                        if j >= ncall:
                            continue
                        g = pool.tile([P, CPC, D], f32, tag=f"g{half}")
                        w = CALL_IDX // 16
                        nc.gpsimd.dma_gather(
                            g[:], src_ap, idxt[:, j * w:(j + 1) * w],
                            CALL_IDX, CALL_IDX, D,
                        )
                        gts[half].append(g)
                return gts["A"], gts["B"]

            def gather_round_fake(tok_ap_a, tok_ap_b):
                gts = {"A": [], "B": []}
                for half, ncall, pool in (("A", ncallA, gap), ("B", ncallB, gbp)):
                    for j in range(ncall):
                        g = pool.tile([P, CPC, D], f32, tag=f"g{half}")
                        nc.vector.memset(g[:], 0.0)
                        gts[half].append(g)
                return gts["A"], gts["B"]

            def scatter_mlp_round(r, layers, gA, gB):
                """Segment-sum + transpose into sT; then per layer in `layers`
                run the MLP producing act tiles; returns (sT, acts)."""
                sT = stp.tile([D, tslot], f32, tag="sT")
                for t in range(ntile):
                    psumT = ps_s.tile([D, P], f32, tag="scat")
                    nc.tensor.matmul(psumT[:], lhsT=ident[:], rhs=zrhs[:],
                                     start=True, stop=False,
                                     skip_group_check=True)
                    nch = len(pl.sched[t]["A"]) + len(pl.sched[t]["B"])
                    ci = 0
                    for half, gts in (("A", gA), ("B", gB)):
                        for (rr, m, dh, q) in pl.sched[t][half]:
                            off = pl.pat_offsets[(dh, m)]
                            g = gts[q // CPC]
                            ci += 1
                            nc.tensor.matmul(
                                psumT[:, rr:rr + m],
                                lhsT=g[:, q % CPC, :],
                                rhs=pat[:, off:off + m],
                                start=False,
                                stop=(ci == nch),
                                skip_group_check=True,
                            )
                    nc.vector.tensor_copy(sT[:, t * P:(t + 1) * P], psumT[:])

                acts = []
                ngrp = (tslot + 511) // 512
                for l in layers:
                    u = actp.tile([D, tslot], f32, tag=f"act{l if r == 3 else 3}")
                    for gi in range(ngrp):
                        c0 = gi * 512
                        c1 = min(c0 + 512, tslot)
                        w = c1 - c0
                        p1 = ps_mm.tile([D, 512], f32, tag="p1")
                        nc.tensor.matmul(p1[:, :w], lhsT=w1sb[l][:],
                                         rhs=sT[:, c0:c1], start=True, stop=True)
                        rl = relup.tile([D, 512], f32, tag="rl")
                        nc.scalar.activation(rl[:, :w], p1[:, :w],
                                             mybir.ActivationFunctionType.Relu,
                                             bias=b1sb[:, l:l + 1])
                        p2 = ps_mm.tile([D, 512], f32, tag="p2")
                        nc.tensor.matmul(p2[:, :w], lhsT=w2sb[l][:],
                                         rhs=rl[:, :w], start=True, stop=True)
                        func = (mybir.ActivationFunctionType.Tanh if r == 3
                                else mybir.ActivationFunctionType.Relu)
                        nc.scalar.activation(u[:, c0:c1], p2[:, :w], func,
                                             bias=b2sb[:, l:l + 1])
                    if rslot < tslot:
                        nc.vector.memset(u[:, rslot:tslot], 0.0)
                    acts.append(u)
                return sT, acts

            def bn_stats_and_norm(r, layers, acts, sT):
                """stats -> AllReduce -> per-layer (a, b) -> normalize in place
                (only cols [0, rslot))."""
                stats = smallp.tile([D, 16], f32, tag="stats")
                nc.vector.memset(stats[:], 0.0)
                for i, l in enumerate(layers):
                    u = acts[i]
                    nc.vector.tensor_reduce(stats[:, 2 * i:2 * i + 1], u[:],
                                            axis=mybir.AxisListType.X,
                                            op=mybir.AluOpType.add)
                    # sum of squares; dump squared values into sT (dead now)
                    nc.vector.tensor_mul(sT[:, :tslot], u[:], u[:])
                    nc.vector.tensor_reduce(stats[:, 2 * i + 1:2 * i + 2],
                                            sT[:, :tslot],
                                            axis=mybir.AxisListType.X,
                                            op=mybir.AluOpType.add)
                ar_in = dramp.tile([D, 16], f32, tag="arin")
                ar_out = dramp.tile([D, 16], f32, tag="arout")
                nc.sync.dma_start(ar_in[:], stats[:])
                if _NOCOLL:
                    nc.gpsimd.dma_start(ar_out[:], ar_in[:])
                else:
                    nc.gpsimd.collective_compute(
                        "AllReduce", mybir.AluOpType.add,
                        replica_groups=[list(range(NCORES))],
                        ins=[ar_in.opt()], outs=[ar_out.opt()],
                    )
                gstats = smallp.tile([D, 16], f32, tag="gstats")
                nc.sync.dma_start(gstats[:], ar_out[:])
                inv_n = 1.0 / pl.n_nodes
                for i, l in enumerate(layers):
                    u = acts[i]
                    mean = smallp.tile([D, 1], f32, tag="mean")
                    nc.vector.tensor_scalar_mul(mean[:], gstats[:, 2 * i:2 * i + 1],
                                                inv_n)
                    var = smallp.tile([D, 1], f32, tag="var")
                    # var = S2/n - mean^2
                    nc.vector.tensor_scalar_mul(var[:],
                                                gstats[:, 2 * i + 1:2 * i + 2],
                                                inv_n)
                    msq = smallp.tile([D, 1], f32, tag="msq")
                    nc.vector.tensor_mul(msq[:], mean[:], mean[:])
                    nc.vector.tensor_sub(var[:], var[:], msq[:])
                    sd = smallp.tile([D, 1], f32, tag="sd")
                    nc.scalar.activation(sd[:], var[:],
                                         mybir.ActivationFunctionType.Sqrt,
                                         bias=epsc[:])
                    rs = smallp.tile([D, 1], f32, tag="rs")
                    nc.vector.reciprocal(rs[:], sd[:])
                    a = smallp.tile([D, 1], f32, tag="a")
                    nc.vector.tensor_mul(a[:], gsb[:, l:l + 1], rs[:])
                    b = smallp.tile([D, 1], f32, tag="b")
                    nc.vector.tensor_mul(b[:], mean[:], a[:])
                    nc.vector.tensor_sub(b[:], btsb[:, l:l + 1], b[:])
                    nc.vector.tensor_scalar(
                        out=u[:, :rslot], in0=u[:, :rslot],
                        scalar1=a[:], scalar2=b[:],
                        op0=mybir.AluOpType.mult, op1=mybir.AluOpType.add)

            # ------------- rounds -------------
            for _rep in range(_REPS):
              for r in range(4):
                  if r >= _ROUNDS and r < 3:
                      continue
                  if _ROUNDS <= 3 and r == 3 and _ROUNDS != 4:
                      # still run round 3 so outputs exist, unless ROUNDS==0
                      pass
                  layers = [r] if r < 3 else [3, 4, 5, 6]
                  if r == 0:
                      src_a = tok0[:split, :]
                      src_b = tok0[split:, :]
                  else:
                      src_a = tok_cur[:split, :]
                      src_b = tok_cur[split:, :]
                  if _NOGATHER:
                      gA, gB = gather_round_fake(src_a, src_b)
                  else:
                      gA, gB = gather_round(src_a, src_b)
                  if _PHASE <= 1:
                      fin = actp.tile([D, tslot], f32, tag="act3")
                      nc.vector.memset(fin[:], 0.0)
                      for gg in (gA + gB)[:1]:
                          nc.vector.tensor_copy(fin[:, :D], gg[:, 0, :])
                      for i in range(4):
                          nc.sync.dma_start(out_d[i, :, :], fin[:])
                      break
                  sT, acts = scatter_mlp_round(r, layers, gA, gB)
                  if _PHASE <= 2:
                      for i in range(4):
                          nc.sync.dma_start(out_d[i, :, :], sT[:])
                      break
                  if _PHASE <= 3:
                      for i in range(4):
                          nc.sync.dma_start(out_d[i, :, :], acts[min(i, len(acts) - 1)][:])
                      break
                  bn_stats_and_norm(r, layers, acts, sT)
                  if _PHASE <= 4:
                      for i in range(4):
                          nc.sync.dma_start(out_d[i, :, :], acts[min(i, len(acts) - 1)][:])
                      break

                  if r < 3:
                      x = acts[0]
                      # transpose x (feature-major) back to node-major token rows
                      ag_in = dramp.tile([tslot, D], f32, tag="agin")
                      for t0 in range(0, ntile, 4):
                          tn = min(4, ntile - t0)
                          psT = ps_t.tile([D, 4 * P], f32, tag="psTw")
                          for tt in range(tn):
                              nc.tensor.transpose(
                                  psT[:, tt * P:(tt + 1) * P],
                                  x[:, (t0 + tt) * P:(t0 + tt + 1) * P],
                                  ident[:])
                          rows = tokp.tile([P, 4 * P], f32, tag="rows")
                          nc.vector.tensor_copy(rows[:, :tn * P], psT[:, :tn * P])
                          # rows[p, tt*P + f] -> ag_in[(t0+tt)*P + p, f]
                          nc.sync.dma_start(
                              ag_in[t0 * P:(t0 + tn) * P, :]
                              .rearrange("(t p) f -> p t f", p=P),
                              rows[:, :tn * P].rearrange("p (t f) -> p t f", f=P))
                      ag_out = dramp.tile([tot, D], f32, tag="agout")
                      if _NOCOLL:
                          nc.gpsimd.dma_start(ag_out[:tslot, :], ag_in[:])
                      else:
                          nc.gpsimd.collective_compute(
                              "AllGather", mybir.AluOpType.bypass,
                              replica_groups=[list(range(NCORES))],
                              ins=[ag_in.opt()], outs=[ag_out.opt()],
                          )
                      tok_cur = ag_out
                  else:
                      for i in range(4):
                          nc.sync.dma_start(out_d[i, :, :], acts[i][:])

    nc.compile()
    return nc


# ----------------------------------------------------------------------------
# runner
# ----------------------------------------------------------------------------

_CACHE = {}


def _get_runner(edge_index: np.ndarray, n_nodes: int):
    key = ("k", edge_index.shape[1], n_nodes, int(edge_index[0, 0]),
           int(edge_index.sum() % (1 << 31)))
    if key in _CACHE:
        return _CACHE[key]
    pl = build_plan(edge_index, n_nodes)
    nc = build_bass(pl)
    _CACHE[key] = (pl, nc)
    return pl, nc


def make_in_maps(pl: Plan, nc, x, W1, b1, W2, b2, gamma, beta):
    tok0 = np.zeros((pl.tot, D), np.float32)
    for c in range(NCORES):
        o = pl.orig_by_core[c]
        tok0[c * pl.tslot: c * pl.tslot + len(o)] = x[o]
    base = {
        "tok0": tok0,
        "w1": np.ascontiguousarray(W1, np.float32),
        "w2": np.ascontiguousarray(W2, np.float32),
        "b1t": np.ascontiguousarray(b1.T, np.float32),
        "b2t": np.ascontiguousarray(b2.T, np.float32),
        "gt": np.ascontiguousarray(gamma.T, np.float32),
        "bt": np.ascontiguousarray(beta.T, np.float32),
        "pat": pl.packmat,
        "ident": np.eye(P, dtype=np.float32),
    }
    return [
        {**base, "idxa": pl.idxA[c], "idxb": pl.idxB[c]}
        for c in range(NCORES)
    ]


def unshard(pl: Plan, results) -> np.ndarray:
    out = np.empty((4, pl.n_nodes, D), np.float32)
    for c in range(NCORES):
        o = pl.orig_by_core[c]
        oc = results[c]["out"]  # [4, D, tslot]
        out[:, o, :] = oc[:, :, :len(o)].transpose(0, 2, 1)
    return out


def _build_sharded(nc, donate: bool):
    """Replicate bass2jax.run_bass_via_pjrt's jit construction so the
    executable can be invoked repeatedly (for timing)."""
    import jax
    import numpy as np_
    from jax.sharding import Mesh, PartitionSpec
    from jax.experimental.shard_map import shard_map
    from concourse import bass2jax
    import concourse.mybir as mybir_
    bass2jax.install_neuronx_cc_hook()
    partition_name = nc.partition_id_tensor.name if nc.partition_id_tensor else None
    in_names, out_names, out_avals, zero_outs = [], [], [], []
    for alloc in nc.m.functions[0].allocations:
        if not isinstance(alloc, mybir_.MemoryLocationSet):
            continue
        name = alloc.memorylocations[0].name
        if alloc.kind == "ExternalInput":
            if name != partition_name:
                in_names.append(name)
        elif alloc.kind == "ExternalOutput":
            out_names.append(name)
            shape = tuple(alloc.tensor_shape)
            dtype = mybir_.dt.np(alloc.dtype)
            out_avals.append(jax.core.ShapedArray(shape, dtype))
            zero_outs.append(np_.zeros(shape, dtype))
    n_params = len(in_names)
    in_names = in_names + out_names
    if partition_name is not None:
        in_names.append(partition_name)

    def _body(*args):
        operands = list(args)
        if partition_name is not None:
            operands.append(bass2jax.partition_id_tensor())
        outs = bass2jax._bass_exec_p.bind(
            *operands,
            out_avals=tuple(out_avals),
            in_names=tuple(in_names),
            out_names=tuple(out_names),
            lowering_input_output_aliases=(),
            sim_require_finite=True,
            sim_require_nnan=True,
            nc=nc,
        )
        return tuple(outs)

    devices = jax.devices()[:NCORES]
    mesh = Mesh(np_.asarray(devices), ("core",))
    n_outs = len(out_names)
    in_specs = (PartitionSpec("core"),) * (n_params + n_outs)
    out_specs = (PartitionSpec("core"),) * n_outs
    kw = dict(donate_argnums=tuple(range(n_params, n_params + n_outs)),
              keep_unused=True) if donate else dict(keep_unused=True)
    sharded = jax.jit(
        shard_map(_body, mesh=mesh, in_specs=in_specs, out_specs=out_specs,
                  check_rep=False), **kw)
    return sharded, in_names[:n_params], out_names, out_avals, zero_outs


def time_kernel(x, edge_index, batch, W1, b1, W2, b2, gamma, beta, iters=20):
    """Return best wall-clock ns of the on-device execution (steady state)."""
    import time as _time
    import jax
    x = np.asarray(x, np.float32)
    edge_index = np.asarray(edge_index, np.int64)
    pl, nc = _get_runner(edge_index, x.shape[0])
    in_maps = make_in_maps(pl, nc, x, W1, b1, W2, b2, gamma, beta)
    sharded, in_names, out_names, out_avals, zero_outs = _build_sharded(nc, donate=False)
    concat_in = [
        np.concatenate([np.asarray(in_maps[c][n]) for c in range(NCORES)], axis=0)
        for n in in_names
    ]
    concat_zeros = [np.zeros((NCORES * z.shape[0], *z.shape[1:]), z.dtype)
                    for z in zero_outs]
    dev_args = [jax.device_put(a) for a in concat_in + concat_zeros]
    outs = sharded(*dev_args)  # compile + warmup
    jax.block_until_ready(outs)
    best = float("inf")
    for _ in range(iters):
        t0 = _time.perf_counter()
        outs = sharded(*dev_args)
        jax.block_until_ready(outs)
        best = min(best, _time.perf_counter() - t0)
    return best * 1e9


def kernel(x, edge_index, batch, W1, b1, W2, b2, gamma, beta):
    from concourse.bass_utils import run_bass_kernel_spmd
    x = np.asarray(x, np.float32)
    edge_index = np.asarray(edge_index, np.int64)
    pl, nc = _get_runner(edge_index, x.shape[0])
    in_maps = make_in_maps(pl, nc, x, W1, b1, W2, b2, gamma, beta)
    res = run_bass_kernel_spmd(nc, in_maps, core_ids=list(range(NCORES)))
    return unshard(pl, res.results)



# revision 12
# speedup vs baseline: 54.2362x; 54.2362x over previous
"""GIN-style 7-layer GNN encoder on 8 Trainium2 NeuronCores.

Self-contained: kernel(**inputs) takes full numpy inputs, shards across 8
cores internally, runs a Bass/Tile kernel via run_bass_kernel_spmd, and
returns the full (4, 50000, 128) float32 output.

Architecture (per core, identical SPMD program; per-core data differs):
  - Tokens (node activations) live in HBM as fp16 [50176, 128] rows (256B),
    split at slot 31360 into halves A (cores 0-4) / B (cores 5-7) so
    dma_gather indices fit in int16.
  - Node->slot assignment: nodes are split randomly into the A/B halves,
    sorted descending by per-half in-degree (nA, nB), and dealt into
    position groups of 8 (5 A-nodes -> cores 0-4, 3 B -> cores 5-7), so the
    cross-core max degree per position ~= mean degree.
  - The gather schedule packs per-position degree runs into exact 128-slot
    groups (positions split across group boundaries), so gather descriptor
    count ~= sum of per-position max degrees (~76.5k/core/round). Gathers
    round-robin over 4 SWDGE queues to overlap Q7 descriptor emission.
  - The GIN self term never goes through the gather: each tile's PSUM
    accumulation starts with identity @ prev-activation (feature-major,
    already in SBUF).
  - BatchNorm folding: rounds 0-2 write back UNNORMALIZED relu activations
    immediately after the MLP (so AllGather + the next round's gathers
    start early); the BN affine (a, b) is folded into the NEXT round's
    first matmul (W1 rows scaled by a; per-node (1+deg)*(W1^T b) added via
    a K=1 matmul against a degree row). The BN stats AllReduce thus
    overlaps the next round's gather phase. Round 3 (layers 3-6) applies
    real BN to the tanh outputs.
  - MLP matmuls run in fp32r (1 cyc/row at 512 cols, near-fp32 accuracy);
    scatter matmuls in fp16 against cached one-hot staircase patterns.
"""

import os

import numpy as np

NCORES = 8
P = 128
D = 128
BN_EPS = 1e-5
CALL_IDX = 1024               # indices per dma_gather call
CPC = CALL_IDX // P           # 128-slot groups per call
NHALF_A = 5                   # cores 0-4 are half A


# ----------------------------------------------------------------------------
# host-side schedule construction
# ----------------------------------------------------------------------------

class Plan:
    pass


def _build_half_schedule(dmax, ntile):
    """Pack per-position degree runs into exact 128-slot groups.

    Returns (chunks_by_tile, ngroups). Each chunk is a dict with
    tile, group, rr (start column in the tile), members=[(pos, consumed,
    take, off)] where `off` is the member's row offset inside the group.
    """
    chunks_by_tile = [[] for _ in range(ntile)]
    group, off = 0, 0
    cur = None
    last_pos = None
    for pos in range(len(dmax)):
        d = int(dmax[pos])
        if d == 0:
            continue
        t = pos // P
        if last_pos is not None and pos != last_pos + 1:
            cur = None                      # non-consecutive: new chunk
        last_pos = pos
        consumed = 0
        while consumed < d:
            if off == P:
                group += 1
                off = 0
                cur = None
            take = min(d - consumed, P - off)
            if cur is None or cur["group"] != group or cur["tile"] != t:
                cur = {"tile": t, "group": group, "rr": pos % P, "members": []}
                chunks_by_tile[t].append(cur)
            cur["members"].append((pos, consumed, take, off))
            off += take
            consumed += take
    ngroups = group + (1 if off > 0 else 0)
    return chunks_by_tile, ngroups


def build_plan(edge_index: np.ndarray, n_nodes: int) -> Plan:
    pl = Plan()
    src = edge_index[0].astype(np.int64)
    dst = edge_index[1].astype(np.int64)

    rslot = int(np.ceil(n_nodes / NCORES))          # real nodes per core
    tslot = int(np.ceil(rslot / P)) * P             # padded slots per core
    ntile = tslot // P
    tot = NCORES * tslot
    split = NHALF_A * tslot                         # half boundary
    assert split < 32768 and tot - split < 32768, (split, tot)

    rng = np.random.RandomState(12345)
    halfA = np.zeros(n_nodes, bool)
    halfA[rng.choice(n_nodes, NHALF_A * rslot, replace=False)] = True

    deg = np.bincount(dst, minlength=n_nodes)       # edge in-degree (no self)
    a_in = np.bincount(dst[halfA[src]], minlength=n_nodes)
    nA = a_in                                       # pure edge counts
    nB = deg - a_in

    A_nodes = np.where(halfA)[0]
    B_nodes = np.where(~halfA)[0]

    def sort_desc(nodes):
        return nodes[np.lexsort((nB[nodes], nA[nodes]))][::-1]

    A_s = sort_desc(A_nodes)
    B_s = sort_desc(B_nodes)
    GA = A_s.reshape(rslot, NHALF_A)                # position i -> cores 0-4
    GB = B_s.reshape(rslot, NCORES - NHALF_A)       # position i -> cores 5-7

    core_of = np.empty(n_nodes, np.int64)
    local_of = np.empty(n_nodes, np.int64)
    for j in range(NHALF_A):
        core_of[GA[:, j]] = j
        local_of[GA[:, j]] = np.arange(rslot)
    for j in range(NCORES - NHALF_A):
        core_of[GB[:, j]] = NHALF_A + j
        local_of[GB[:, j]] = np.arange(rslot)
    slot_of = core_of * tslot + local_of

    pl.orig_by_core = [np.where(core_of == c)[0][np.argsort(local_of[core_of == c])]
                       for c in range(NCORES)]

    dmaxA = np.zeros(tslot, np.int64)
    dmaxA[:rslot] = np.maximum(nA[GA].max(axis=1), nA[GB].max(axis=1))
    dmaxB = np.zeros(tslot, np.int64)
    dmaxB[:rslot] = np.maximum(nB[GA].max(axis=1), nB[GB].max(axis=1))
    assert dmaxA.max() <= P and dmaxB.max() <= P

    # edge CSR by global dst slot; src as slots
    dst_slot = slot_of[dst]
    src_slot = slot_of[src]
    sort_idx = np.argsort(dst_slot, kind="stable")
    ds_sorted = dst_slot[sort_idx]
    ss_sorted = src_slot[sort_idx]
    cnt = np.bincount(ds_sorted, minlength=tot)
    offs = np.concatenate([[0], np.cumsum(cnt)])

    chunksA, ngA = _build_half_schedule(dmaxA, ntile)
    chunksB, ngB = _build_half_schedule(dmaxB, ntile)
    pl.ngroups = {"A": ngA, "B": ngB}
    pl.ncalls = {"A": int(np.ceil(ngA / CPC)), "B": int(np.ceil(ngB / CPC))}

    # pattern matrix: one column per chunk member; ones at group-absolute rows
    pat_cols = []
    sched = []
    for t in range(ntile):
        ent = {}
        for half, cbt in (("A", chunksA), ("B", chunksB)):
            lst = []
            for ch in cbt[t]:
                po = len(pat_cols)
                for (pos, consumed, take, off) in ch["members"]:
                    pat_cols.append((off, take))
                g = ch["group"]
                lst.append((ch["rr"], len(ch["members"]), po, g // CPC, g % CPC))
            ent[half] = lst
        sched.append(ent)
    pl.sched = sched
    pl.pat_total_cols = len(pat_cols)
    packmat = np.zeros((P, len(pat_cols)), np.float32)
    for j, (off, take) in enumerate(pat_cols):
        packmat[off:off + take, j] = 1.0
    pl.packmat = packmat
    pl.nchunks = {"A": sum(len(c) for c in chunksA),
                  "B": sum(len(c) for c in chunksB)}

    # per-core gather index arrays
    zslot_a = rslot                                 # a pad slot of core 0
    zslot_b = NHALF_A * tslot + rslot - split       # pad slot of core 5, rebased
    pl.idxA, pl.idxB = [], []
    for c in range(NCORES):
        base_slot = c * tslot
        for half, cbt, zs, ncall, outlist in (
            ("A", chunksA, zslot_a, pl.ncalls["A"], pl.idxA),
            ("B", chunksB, zslot_b, pl.ncalls["B"], pl.idxB),
        ):
            rebase = 0 if half == "A" else split
            want_a = half == "A"
            flat = np.full(ncall * CALL_IDX, zs, np.int16)
            for t in range(ntile):
                for ch in cbt[t]:
                    g = ch["group"]
                    for (pos, consumed, take, off) in ch["members"]:
                        if pos >= rslot:
                            continue
                        gslot = base_slot + pos
                        e0, e1 = offs[gslot], offs[gslot + 1]
                        ss = ss_sorted[e0:e1]
                        ss = ss[(ss < split) == want_a] - rebase
                        avail = ss[consumed:consumed + take]
                        b = g * P + off
                        flat[b:b + len(avail)] = avail
            a = flat.reshape(-1, 16).T
            outlist.append(np.tile(a, (8, 1)).astype(np.int16))

    # per-core degree rows: (1 + full edge in-degree) by position, pads 0
    pl.degp = []
    for c in range(NCORES):
        o = pl.orig_by_core[c]
        dp = np.zeros((1, tslot), np.float16)
        dp[0, :len(o)] = (1.0 + deg[o]).astype(np.float16)
        pl.degp.append(dp)

    pl.rslot, pl.tslot, pl.ntile, pl.tot, pl.split = rslot, tslot, ntile, tot, split
    pl.n_nodes = n_nodes
    pl.inflation = (pl.ncalls["A"] + pl.ncalls["B"]) * CALL_IDX * NCORES / max(len(src), 1)
    return pl


# ----------------------------------------------------------------------------
# bass kernel builder
# ----------------------------------------------------------------------------

def build_bass(pl: Plan):
    _NOCOLL = bool(os.environ.get("KDBG_NOCOLL"))
    _ROUNDS = int(os.environ.get("KDBG_ROUNDS", "4"))
    _NOGATHER = bool(os.environ.get("KDBG_NOGATHER"))
    _PHASE = int(os.environ.get("KDBG_PHASE", "99"))
    _REPS = int(os.environ.get("KDBG_REPS", "1"))
    _QUEUES = int(os.environ.get("KDBG_QUEUES", "4"))
    import concourse.bacc as bacc
    import concourse.bass as bass
    import concourse.mybir as mybir
    import concourse.tile as tile
    from concourse.library_config import mlp as mlp_lib

    f32 = mybir.dt.float32
    f32r = mybir.dt.float32r
    f16 = mybir.dt.float16
    i16 = mybir.dt.int16
    tslot, ntile, tot, split = pl.tslot, pl.ntile, pl.tot, pl.split
    rslot = pl.rslot
    ncallA, ncallB = pl.ncalls["A"], pl.ncalls["B"]
    NL = 7

    nc = bacc.Bacc("TRN2", target_bir_lowering=False, debug=False,
                   num_devices=NCORES, num_swdge_queues=max(_QUEUES, 1))

    tok0 = nc.dram_tensor("tok0", [tot, D], f16, kind="ExternalInput")
    xown_d = nc.dram_tensor("xown", [tslot, D], f16, kind="ExternalInput")
    degp_d = nc.dram_tensor("degp", [1, tslot], f16, kind="ExternalInput")
    w1 = nc.dram_tensor("w1", [NL, D, D], f32r, kind="ExternalInput")
    w2 = nc.dram_tensor("w2", [NL, D, D], f32r, kind="ExternalInput")
    b1t = nc.dram_tensor("b1t", [D, NL], f32, kind="ExternalInput")
    b2t = nc.dram_tensor("b2t", [D, NL], f32, kind="ExternalInput")
    gt = nc.dram_tensor("gt", [D, NL], f32, kind="ExternalInput")
    bt = nc.dram_tensor("bt", [D, NL], f32, kind="ExternalInput")
    idxa_d = nc.dram_tensor("idxa", [128, ncallA * (CALL_IDX // 16)], i16,
                            kind="ExternalInput")
    idxb_d = nc.dram_tensor("idxb", [128, ncallB * (CALL_IDX // 16)], i16,
                            kind="ExternalInput")
    pat_d = nc.dram_tensor("pat", [P, pl.pat_total_cols], f16,
                           kind="ExternalInput")
    ident_d = nc.dram_tensor("ident", [P, P], f16, kind="ExternalInput")
    out_d = nc.dram_tensor("out", [4, D, tslot], f32, kind="ExternalOutput")

    with tile.TileContext(nc) as tc:
        nc.gpsimd.load_library(mlp_lib)
        with (
            tc.tile_pool(name="const", bufs=1) as constp,
            tc.tile_pool(name="ga", bufs=6) as gap,
            tc.tile_pool(name="gb", bufs=6) as gbp,
            tc.tile_pool(name="st", bufs=1) as stp,
            tc.tile_pool(name="act", bufs=1) as actp,
            tc.tile_pool(name="relu", bufs=2) as relup,
            tc.tile_pool(name="fin", bufs=1) as finp,
            tc.tile_pool(name="toks", bufs=2) as tokp,
            tc.tile_pool(name="small", bufs=8) as smallp,
            tc.tile_pool(name="w1p", bufs=1) as w1pp,
            tc.tile_pool(name="ps_s", bufs=3, space="PSUM") as ps_s,
            tc.tile_pool(name="ps_t", bufs=2, space="PSUM") as ps_t,
            tc.tile_pool(name="ps_mm", bufs=1, space="PSUM") as ps_mm,
            tc.tile_pool(name="dram", bufs=2, space="DRAM") as dramp,
        ):
            # ---- constants ----
            ident = constp.tile([P, P], f16)
            nc.sync.dma_start(ident[:], ident_d[:, :])
            pat = constp.tile([P, pl.pat_total_cols], f16)
            nc.sync.dma_start(pat[:], pat_d[:, :])
            idxa = constp.tile([128, ncallA * (CALL_IDX // 16)], i16)
            nc.sync.dma_start(idxa[:], idxa_d[:, :])
            idxb = constp.tile([128, ncallB * (CALL_IDX // 16)], i16)
            nc.sync.dma_start(idxb[:], idxb_d[:, :])
            degp = constp.tile([1, tslot], f16)
            nc.sync.dma_start(degp[:], degp_d[:, :])
            w1sb = [constp.tile([D, D], f32r, name=f"w1_{l}", tag=f"w1_{l}") for l in range(NL)]
            w2sb = [constp.tile([D, D], f32r, name=f"w2_{l}", tag=f"w2_{l}") for l in range(NL)]
            for l in range(NL):
                nc.sync.dma_start(w1sb[l][:], w1[l, :, :])
                nc.sync.dma_start(w2sb[l][:], w2[l, :, :])
            b1sb = constp.tile([D, NL], f32)
            nc.sync.dma_start(b1sb[:], b1t[:, :])
            b2sb = constp.tile([D, NL], f32)
            nc.sync.dma_start(b2sb[:], b2t[:, :])
            gsb = constp.tile([D, NL], f32)
            nc.sync.dma_start(gsb[:], gt[:, :])
            btsb = constp.tile([D, NL], f32)
            nc.sync.dma_start(btsb[:], bt[:, :])
            epsc = constp.tile([D, 1], f32)
            nc.vector.memset(epsc[:], BN_EPS)
            # round-0 self term: own tokens, transposed to feature-major
            uprev0 = constp.tile([D, tslot], f16, name="uprev0")
            nc.sync.dma_start_transpose(uprev0[:], xown_d[:, :])

            qctr = [0]

            def gather_round(tok_ap_a, tok_ap_b):
                gts = {"A": [], "B": []}
                for j in range(max(ncallA, ncallB)):
                    for half, ncall, idxt, src_ap, pool in (
                        ("A", ncallA, idxa, tok_ap_a, gap),
                        ("B", ncallB, idxb, tok_ap_b, gbp),
                    ):
                        if j >= ncall:
                            continue
                        g = pool.tile([P, CPC, D], f16, tag=f"g{half}")
                        w = CALL_IDX // 16
                        nc.gpsimd.dma_gather(
                            g[:], src_ap, idxt[:, j * w:(j + 1) * w],
                            CALL_IDX, CALL_IDX, D,
                            queue_num=(qctr[0] % _QUEUES),
                        )
                        qctr[0] += 1
                        gts[half].append(g)
                return gts["A"], gts["B"]

            def gather_round_fake(tok_ap_a, tok_ap_b):
                gts = {"A": [], "B": []}
                for half, ncall, pool in (("A", ncallA, gap), ("B", ncallB, gbp)):
                    for j in range(ncall):
                        g = pool.tile([P, CPC, D], f16, tag=f"g{half}")
                        nc.vector.memset(g[:], 0.0)
                        gts[half].append(g)
                return gts["A"], gts["B"]

            def scatter_mlp_round(r, layers, gA, gB, uprev, fold):
                """Segment-sum (self term via identity matmul on uprev) into
                sT, then per layer run the (BN-folded) MLP."""
                sT = stp.tile([D, tslot], f32r, tag="sT")
                for t in range(ntile):
                    psumT = ps_s.tile([D, P], f32, tag="scat")
                    nc.tensor.matmul(psumT[:], lhsT=ident[:],
                                     rhs=uprev[:, t * P:(t + 1) * P],
                                     start=True, stop=False,
                                     skip_group_check=True)
                    nch = len(pl.sched[t]["A"]) + len(pl.sched[t]["B"])
                    ci = 0
                    for half, gts in (("A", gA), ("B", gB)):
                        for (rr, m, po, call, sub) in pl.sched[t][half]:
                            g = gts[call]
                            ci += 1
                            nc.tensor.matmul(
                                psumT[:, rr:rr + m],
                                lhsT=g[:, sub, :],
                                rhs=pat[:, po:po + m],
                                start=False,
                                stop=(ci == nch),
                                skip_group_check=True,
                            )
                    nc.vector.tensor_copy(sT[:, t * P:(t + 1) * P], psumT[:])

                acts = []
                ngrp = (tslot + 511) // 512
                for l in layers:
                    if fold is None:
                        w1l = w1sb[l]
                        vbT = None
                    else:
                        a_prev, b_prev = fold
                        w1l = w1pp.tile([D, D], f32r, tag=f"w1p_{l}")
                        nc.vector.tensor_scalar(
                            out=w1l[:], in0=w1sb[l][:], scalar1=a_prev[:],
                            scalar2=None, op0=mybir.AluOpType.mult)
                        # vb = W1^T b_prev ; vbT = its transpose [1, D] fp16
                        # (fp32r matmuls need even column counts: N=2)
                        vb = ps_t.tile([D, 2], f32, tag="psTw")
                        nc.tensor.matmul(vb[:, :2], lhsT=w1sb[l][:],
                                         rhs=b_prev[:, :2], start=True, stop=True)
                        vb16 = smallp.tile([D, 1], f16, tag="vb16")
                        nc.vector.tensor_copy(vb16[:], vb[:, :1])
                        vbt_ps = ps_t.tile([1, P], f16, tag="psTw")
                        nc.tensor.transpose(vbt_ps[:], vb16[:], ident[:])
                        vbT = smallp.tile([1, P], f16, tag="vbT")
                        nc.vector.tensor_copy(vbT[:], vbt_ps[:])
                    u = actp.tile([D, tslot], f16, tag=f"act{l if r == 3 else 3}")
                    for gi in range(ngrp):
                        c0 = gi * 512
                        c1 = min(c0 + 512, tslot)
                        w = c1 - c0
                        p1 = ps_mm.tile([D, 512], f32, tag="p1")
                        nc.tensor.matmul(p1[:, :w], lhsT=w1l[:],
                                         rhs=sT[:, c0:c1],
                                         start=True, stop=(vbT is None))
                        if vbT is not None:
                            nc.tensor.matmul(p1[:, :w], lhsT=vbT[:, :],
                                             rhs=degp[:, c0:c1],
                                             start=False, stop=True,
                                             skip_group_check=True)
                        rl = relup.tile([D, 512], f32r, tag="rl")
                        nc.scalar.activation(rl[:, :w], p1[:, :w],
                                             mybir.ActivationFunctionType.Relu,
                                             bias=b1sb[:, l:l + 1])
                        p2 = ps_mm.tile([D, 512], f32, tag="p2")
                        nc.tensor.matmul(p2[:, :w], lhsT=w2sb[l][:],
                                         rhs=rl[:, :w],
                                         start=True, stop=True)
                        func = (mybir.ActivationFunctionType.Tanh if r == 3
                                else mybir.ActivationFunctionType.Relu)
                        nc.scalar.activation(u[:, c0:c1], p2[:, :w], func,
                                             bias=b2sb[:, l:l + 1])
                    if rslot < tslot:
                        nc.vector.memset(u[:, rslot:tslot], 0.0)
                    acts.append(u)
                return sT, acts

            def writeback_allgather(x):
                """Transpose x (feature-major fp16) to node-major token rows,
                DMA to DRAM, AllGather. Returns the gathered token tensor."""
                ag_in = dramp.tile([tslot, D], f16, tag="agin")
                for t0 in range(0, ntile, 4):
                    tn = min(4, ntile - t0)
                    psT = ps_t.tile([D, 4 * P], f16, tag="psTw")
                    for tt in range(tn):
                        nc.tensor.transpose(
                            psT[:, tt * P:(tt + 1) * P],
                            x[:, (t0 + tt) * P:(t0 + tt + 1) * P],
                            ident[:])
                    rows = tokp.tile([P, 4 * P], f16, tag="rows")
                    nc.vector.tensor_copy(rows[:, :tn * P], psT[:, :tn * P])
                    nc.sync.dma_start(
                        ag_in[t0 * P:(t0 + tn) * P, :]
                        .rearrange("(t p) f -> p t f", p=P),
                        rows[:, :tn * P].rearrange("p (t f) -> p t f", f=P))
                ag_out = dramp.tile([tot, D], f16, tag="agout",
                                    addr_space="Shared")
                if _NOCOLL:
                    nc.gpsimd.dma_start(ag_out[:tslot, :], ag_in[:])
                else:
                    nc.gpsimd.collective_compute(
                        "AllGather", mybir.AluOpType.bypass,
                        replica_groups=[list(range(NCORES))],
                        ins=[ag_in.opt()], outs=[ag_out.opt()],
                    )
                return ag_out

            def bn_stats(layers, acts):
                """Per-layer (sum, sumsq) -> AllReduce -> gstats tile."""
                sq = finp.tile([D, tslot], f32, tag="fin")
                stats = smallp.tile([D, 16], f32, tag="stats")
                nc.vector.memset(stats[:], 0.0)
                for i, l in enumerate(layers):
                    u = acts[i]
                    nc.vector.tensor_reduce(stats[:, 2 * i:2 * i + 1], u[:],
                                            axis=mybir.AxisListType.X,
                                            op=mybir.AluOpType.add)
                    nc.vector.tensor_mul(sq[:, :tslot], u[:], u[:])
                    nc.vector.tensor_reduce(stats[:, 2 * i + 1:2 * i + 2],
                                            sq[:, :tslot],
                                            axis=mybir.AxisListType.X,
                                            op=mybir.AluOpType.add)
                ar_in = dramp.tile([D, 16], f32, tag="arin")
                ar_out = dramp.tile([D, 16], f32, tag="arout", addr_space="Shared")
                nc.sync.dma_start(ar_in[:], stats[:])
                if _NOCOLL:
                    nc.gpsimd.dma_start(ar_out[:], ar_in[:])
                else:
                    nc.gpsimd.collective_compute(
                        "AllReduce", mybir.AluOpType.add,
                        replica_groups=[list(range(NCORES))],
                        ins=[ar_in.opt()], outs=[ar_out.opt()],
                    )
                gstats = smallp.tile([D, 16], f32, tag="gstats")
                nc.sync.dma_start(gstats[:], ar_out[:])
                return gstats

            def bn_ab(gstats, i, l):
                """Compute BN affine a = gamma/std, b = beta - mean*a."""
                inv_n = 1.0 / pl.n_nodes
                mean = smallp.tile([D, 1], f32, tag="mean")
                nc.vector.tensor_scalar_mul(mean[:], gstats[:, 2 * i:2 * i + 1],
                                            inv_n)
                var = smallp.tile([D, 1], f32, tag="var")
                nc.vector.tensor_scalar_mul(var[:],
                                            gstats[:, 2 * i + 1:2 * i + 2],
                                            inv_n)
                msq = smallp.tile([D, 1], f32, tag="msq")
                nc.vector.tensor_mul(msq[:], mean[:], mean[:])
                nc.vector.tensor_sub(var[:], var[:], msq[:])
                sd = smallp.tile([D, 1], f32, tag="sd")
                nc.scalar.activation(sd[:], var[:],
                                     mybir.ActivationFunctionType.Sqrt,
                                     bias=epsc[:])
                rs = smallp.tile([D, 1], f32, tag="rs")
                nc.vector.reciprocal(rs[:], sd[:])
                a = smallp.tile([D, 1], f32, tag="a")
                nc.vector.tensor_mul(a[:], gsb[:, l:l + 1], rs[:])
                b = smallp.tile([D, 1], f32, tag="b")
                nc.vector.tensor_mul(b[:], mean[:], a[:])
                nc.vector.tensor_sub(b[:], btsb[:, l:l + 1], b[:])
                return a, b

            # ------------- rounds -------------
            for _rep in range(_REPS):
              uprev = uprev0
              fold = None
              for r in range(4):
                  if r >= _ROUNDS and r < 3:
                      continue
                  layers = [r] if r < 3 else [3, 4, 5, 6]
                  if r == 0:
                      src_a = tok0[:split, :]
                      src_b = tok0[split:, :]
                  else:
                      src_a = tok_cur[:split, :]
                      src_b = tok_cur[split:, :]
                  if _NOGATHER:
                      gA, gB = gather_round_fake(src_a, src_b)
                  else:
                      gA, gB = gather_round(src_a, src_b)
                  sT, acts = scatter_mlp_round(r, layers, gA, gB, uprev, fold)
                  if _PHASE <= 2:
                      for i in range(4):
                          nc.sync.dma_start(out_d[i, :, :], sT[:].bitcast(f32))
                      break
                  if r < 3:
                      tok_cur = writeback_allgather(acts[0])
                      gstats = bn_stats([r], acts)
                      a, b = bn_ab(gstats, 0, r)
                      bfr = smallp.tile([D, 2], f32r, tag="bfr")
                      nc.vector.tensor_copy(bfr[:, :1], b[:])
                      nc.vector.tensor_copy(bfr[:, 1:2], b[:])
                      uprev = acts[0]
                      fold = (a, bfr)
                  else:
                      gstats = bn_stats(layers, acts)
                      for i, l in enumerate(layers):
                          a, b = bn_ab(gstats, i, l)
                          fin = finp.tile([D, tslot], f32, tag="fin")
                          nc.vector.tensor_scalar(
                              out=fin[:, :rslot], in0=acts[i][:, :rslot],
                              scalar1=a[:], scalar2=b[:],
                              op0=mybir.AluOpType.mult, op1=mybir.AluOpType.add)
                          nc.sync.dma_start(out_d[i, :, :], fin[:])

    nc.compile()
    return nc


# ----------------------------------------------------------------------------
# runner
# ----------------------------------------------------------------------------

_CACHE = {}


def _get_runner(edge_index: np.ndarray, n_nodes: int):
    key = ("k", edge_index.shape[1], n_nodes, int(edge_index[0, 0]),
           int(edge_index.sum() % (1 << 31)))
    if key in _CACHE:
        return _CACHE[key]
    pl = build_plan(edge_index, n_nodes)
    nc = build_bass(pl)
    _CACHE[key] = (pl, nc)
    return pl, nc


def _round_f32r(a: np.ndarray) -> np.ndarray:
    """Round fp32 to the fp32r grid (exactly representable as a bf16 pair)."""
    import ml_dtypes
    bf16 = ml_dtypes.bfloat16
    a = np.asarray(a, np.float32)
    hi = a.astype(bf16).astype(np.float32)
    lo = (a - hi).astype(bf16).astype(np.float32)
    return hi + lo


def make_in_maps(pl: Plan, nc, x, W1, b1, W2, b2, gamma, beta):
    tok0 = np.zeros((pl.tot, D), np.float16)
    xb = np.asarray(x, np.float32).astype(np.float16)
    for c in range(NCORES):
        o = pl.orig_by_core[c]
        tok0[c * pl.tslot: c * pl.tslot + len(o)] = xb[o]
    base = {
        "tok0": tok0,
        "w1": np.ascontiguousarray(_round_f32r(W1)),
        "w2": np.ascontiguousarray(_round_f32r(W2)),
        "b1t": np.ascontiguousarray(np.asarray(b1, np.float32).T),
        "b2t": np.ascontiguousarray(np.asarray(b2, np.float32).T),
        "gt": np.ascontiguousarray(np.asarray(gamma, np.float32).T),
        "bt": np.ascontiguousarray(np.asarray(beta, np.float32).T),
        "pat": pl.packmat.astype(np.float16),
        "ident": np.eye(P, dtype=np.float16),
    }
    return [
        {**base, "idxa": pl.idxA[c], "idxb": pl.idxB[c],
         "xown": np.ascontiguousarray(tok0[c * pl.tslot:(c + 1) * pl.tslot]),
         "degp": pl.degp[c]}
        for c in range(NCORES)
    ]


def unshard(pl: Plan, results) -> np.ndarray:
    out = np.empty((4, pl.n_nodes, D), np.float32)
    for c in range(NCORES):
        o = pl.orig_by_core[c]
        oc = results[c]["out"]  # [4, D, tslot]
        out[:, o, :] = oc[:, :, :len(o)].transpose(0, 2, 1)
    return out


def _build_sharded(nc, donate: bool):
    """Replicate bass2jax.run_bass_via_pjrt's jit construction so the
    executable can be invoked repeatedly (for timing)."""
    import jax
    import numpy as np_
    from jax.sharding import Mesh, PartitionSpec
    from jax.experimental.shard_map import shard_map
    from concourse import bass2jax
    import concourse.mybir as mybir_
    bass2jax.install_neuronx_cc_hook()
    partition_name = nc.partition_id_tensor.name if nc.partition_id_tensor else None
    in_names, out_names, out_avals, zero_outs = [], [], [], []
    for alloc in nc.m.functions[0].allocations:
        if not isinstance(alloc, mybir_.MemoryLocationSet):
            continue
        name = alloc.memorylocations[0].name
        if alloc.kind == "ExternalInput":
            if name != partition_name:
                in_names.append(name)
        elif alloc.kind == "ExternalOutput":
            out_names.append(name)
            shape = tuple(alloc.tensor_shape)
            dtype = mybir_.dt.np(alloc.dtype)
            out_avals.append(jax.core.ShapedArray(shape, dtype))
            zero_outs.append(np_.zeros(shape, dtype))
    n_params = len(in_names)
    in_names = in_names + out_names
    if partition_name is not None:
        in_names.append(partition_name)

    def _body(*args):
        operands = list(args)
        if partition_name is not None:
            operands.append(bass2jax.partition_id_tensor())
        outs = bass2jax._bass_exec_p.bind(
            *operands,
            out_avals=tuple(out_avals),
            in_names=tuple(in_names),
            out_names=tuple(out_names),
            lowering_input_output_aliases=(),
            sim_require_finite=True,
            sim_require_nnan=True,
            nc=nc,
        )
        return tuple(outs)

    devices = jax.devices()[:NCORES]
    mesh = Mesh(np_.asarray(devices), ("core",))
    n_outs = len(out_names)
    in_specs = (PartitionSpec("core"),) * (n_params + n_outs)
    out_specs = (PartitionSpec("core"),) * n_outs
    kw = dict(donate_argnums=tuple(range(n_params, n_params + n_outs)),
              keep_unused=True) if donate else dict(keep_unused=True)
    sharded = jax.jit(
        shard_map(_body, mesh=mesh, in_specs=in_specs, out_specs=out_specs,
                  check_rep=False), **kw)
    return sharded, in_names[:n_params], out_names, out_avals, zero_outs


def time_kernel(x, edge_index, batch, W1, b1, W2, b2, gamma, beta, iters=20):
    """Return best wall-clock ns of the on-device execution (steady state)."""
    import time as _time
    import jax
    x = np.asarray(x, np.float32)
    edge_index = np.asarray(edge_index, np.int64)
    pl, nc = _get_runner(edge_index, x.shape[0])
    in_maps = make_in_maps(pl, nc, x, W1, b1, W2, b2, gamma, beta)
    sharded, in_names, out_names, out_avals, zero_outs = _build_sharded(nc, donate=False)
    concat_in = [
        np.concatenate([np.asarray(in_maps[c][n]) for c in range(NCORES)], axis=0)
        for n in in_names
    ]
    concat_zeros = [np.zeros((NCORES * z.shape[0], *z.shape[1:]), z.dtype)
                    for z in zero_outs]
    dev_args = [jax.device_put(a) for a in concat_in + concat_zeros]
    outs = sharded(*dev_args)  # compile + warmup
    jax.block_until_ready(outs)
    best = float("inf")
    for _ in range(iters):
        t0 = _time.perf_counter()
        outs = sharded(*dev_args)
        jax.block_until_ready(outs)
        best = min(best, _time.perf_counter() - t0)
    return best * 1e9


def kernel(x, edge_index, batch, W1, b1, W2, b2, gamma, beta):
    from concourse.bass_utils import run_bass_kernel_spmd
    x = np.asarray(x, np.float32)
    edge_index = np.asarray(edge_index, np.int64)
    pl, nc = _get_runner(edge_index, x.shape[0])
    in_maps = make_in_maps(pl, nc, x, W1, b1, W2, b2, gamma, beta)
    res = run_bass_kernel_spmd(nc, in_maps, core_ids=list(range(NCORES)))
    return unshard(pl, res.results)
